# revision 1
# baseline (speedup 1.0000x reference)
"""Distributed Trainium2 kernel for EnhancedSelfAttention (causal attention
with additive ALiBi |i-j| bias) on 8 NeuronCores.

Math: for queries i and keys j<=i the bias is slope*(i-j), so
softmax_j(S_ij + slope*(i-j)) == softmax_j(S_ij - slope*j) — the slope*i term
is constant per row and cancels. Folding w_j = exp(-slope*j) into V's rows
(plus an appended w column for the denominator) turns the whole softmax into
exp(S) followed by a single PV matmul and a divide. w_j underflows to exactly
0 beyond j ~ 75/slope, so early heads only need the first few key blocks.

Sharding: 8 cores = 2 batches x 4 head groups (4 heads each). Each core
computes its partial projection output; partials are summed on the host.
All cores run one SPMD program: per-slot key-block budgets [16, 16, 10, 3]
with heads assigned to slots so that each head's needed blocks <= budget.

Attention works on S^T tiles ([key, query] layout) so the PV contraction
needs no transposes; exp runs on 1024-wide strips (2 key blocks) to amortize
ACT instruction overhead; the divide broadcasts den (fp16) via a ones-matmul
then applies reciprocal_approx_fast.
"""

import sys
import types

import numpy as np

import concourse.bass as bass
import concourse.mybir as mybir
import concourse.tile as tile
from concourse import bacc
from concourse.bass_utils import run_bass_kernel_spmd


def _ensure_axon_hooks():
    """concourse's trace path imports antenv.axon_hooks, which this image
    lacks; give it a no-op fallback so BASS_TRACE=1 can't crash the run."""
    try:
        import antenv.axon_hooks  # noqa: F401
    except Exception:
        try:
            import antenv
            mod = types.ModuleType("antenv.axon_hooks")
            mod.get_axon_ntff_profile_hook = lambda: None
            mod.set_axon_ntff_profile_hook = lambda h: None
            sys.modules["antenv.axon_hooks"] = mod
            antenv.axon_hooks = mod
        except Exception:
            pass


_ensure_axon_hooks()

F32 = mybir.dt.float32
F16 = mybir.dt.float16
ExpF = mybir.ActivationFunctionType.Exp

B, T, C = 2, 2048, 1024
NH, D = 16, 64
P = 128
NT = T // P            # 16 t tiles
KC = C // P            # 8 contraction subtiles for qkv/proj
QCH = 4                # q chunks of 512
KBUD = (16, 16, 10, 3)  # per-slot key-block budgets
N_CORES = 8

# head -> (group, slot): slot0 gets h8,h10,h12,h14; slot1 h9,h11,h13,h15;
# slot2 h4..h7; slot3 h0..h3.  group g heads:
GROUP_HEADS = [(8 + 2 * g, 9 + 2 * g, 4 + g, g) for g in range(4)]

TRACE = False  # test harness sets kernel.TRACE = True for NTFF profiling

_CACHE = {}


def _slopes():
    i = np.arange(1, NH + 1, dtype=np.float64)
    return (1.0 / np.power(2.0, 8.0 * i / NH)).astype(np.float64)


def _build_program():
    nc = bacc.Bacc("TRN2", target_bir_lowering=False, debug=False,
                   num_devices=N_CORES)

    xt_d = nc.dram_tensor("xt", [C, T], F16, kind="ExternalInput").ap()
    wq_d = nc.dram_tensor("wq", [C, 4 * D], F16, kind="ExternalInput").ap()
    wk_d = nc.dram_tensor("wk", [C, 4 * D], F16, kind="ExternalInput").ap()
    wv_d = nc.dram_tensor("wv", [C, 4 * D], F16, kind="ExternalInput").ap()
    wp_d = nc.dram_tensor("wp", [4 * D, C], F16, kind="ExternalInput").ap()
    wcol_d = nc.dram_tensor("wcol", [T, 4], F32, kind="ExternalInput").ap()
    masks_d = nc.dram_tensor("masks", [P, 4 * 512], F16, kind="ExternalInput").ap()
    y_d = nc.dram_tensor("y", [T, C], F16, kind="ExternalOutput").ap()

    with tile.TileContext(nc) as tc:
        with (
            nc.allow_low_precision(reason="fp16 matmul operands by design"),
            tc.tile_pool(name="const", bufs=1) as const,
            tc.tile_pool(name="psB", bufs=2, space="PSUM") as psB,
            tc.tile_pool(name="psO", bufs=3, space="PSUM") as psO,
            tc.tile_pool(name="psR", bufs=1, space="PSUM") as psR,
            tc.tile_pool(name="pp", bufs=4) as pp,
            tc.tile_pool(name="rr", bufs=3) as rr,
            tc.tile_pool(name="rbp", bufs=3) as rbp,
            tc.tile_pool(name="yp", bufs=4) as yp,
        ):
            # ---- persistent SBUF loads
            # Weights first (small), then xt in nch-major order: a QKV group
            # (m, nch) contracts over all 8 k-chunks but reads only its own
            # 512-column slice, so column-major arrival lets the first group
            # finish after ~2MB instead of the full 4MB.
            wq_sb = const.tile([P, KC * 4 * D], F16, tag="wq")
            wk_sb = const.tile([P, KC * 4 * D], F16, tag="wk")
            wv_sb = const.tile([P, KC * 4 * D], F16, tag="wv")
            for w_sb, w_d in ((wq_sb, wq_d), (wk_sb, wk_d), (wv_sb, wv_d)):
                for k in range(KC):
                    nc.sync.dma_start(w_sb[:, k * 256:(k + 1) * 256],
                                      w_d[k * P:(k + 1) * P, :])
            xt_sb = const.tile([P, KC * T], F16, tag="xt")       # 32KB/part
            # left halves of every k-chunk first: the first QKV groups
            # (nch 0/1) can start after 2MB arrives instead of 4MB
            for h in range(2):
                for k in range(KC):
                    nc.sync.dma_start(
                        xt_sb[:, k * T + h * 1024: k * T + (h + 1) * 1024],
                        xt_d[k * P:(k + 1) * P, h * 1024:(h + 1) * 1024])
            # DMA issue order follows first use: wcol feeds the first V
            # eviction (~27us), masks the first diagonal strip (~65us), wp
            # only the projection (~140us).
            wcol_sb = const.tile([P, NT, 4], F32, tag="wcol")
            nc.sync.dma_start(
                wcol_sb[:], wcol_d.rearrange("(n p) c -> p n c", p=P))
            masks_sb = const.tile([P, 4 * 512], F16, tag="masks")
            nc.sync.dma_start(masks_sb[:], masks_d[:])
            wp_sb = const.tile([P, 2 * C], F16, tag="wp")
            for j in range(2):
                nc.sync.dma_start(wp_sb[:, j * C:(j + 1) * C],
                                  wp_d[j * P:(j + 1) * P, :])
            ones_sb = const.tile([1, D], F16, tag="ones")
            nc.any.memset(ones_sb[:], 1.0)
            # warm the ACT exp table during the DMA wait
            warm_sb = const.tile([1, D], F16, tag="warm")
            nc.scalar.activation(warm_sb[:], ones_sb[:], ExpF)

            qt_sb = [const.tile([P, T], F16, tag=f"qt{m}", name=f"qt{m}")
                     for m in range(2)]
            kt_sb = [const.tile([P, T], F16, tag=f"kt{m}", name=f"kt{m}")
                     for m in range(2)]
            vv_sb = const.tile([P, NT, 4, 65], F16, tag="vv")
            ot_sb = [const.tile([P, T], F16, tag=f"ot{m}", name=f"ot{m}")
                     for m in range(2)]

            # ---- phase 1: QT/KT ([d, t] layout) interleaved with V groups so
            # each group's DVE evictions overlap the other stream's matmuls.
            def emit_qkt_group(i, pool=None):
                w_sb, dst = ((wq_sb, qt_sb), (wk_sb, kt_sb))[i // 8]
                m, nch = divmod(i % 8, QCH)
                if pool is None:
                    ps = psB.tile([P, 1024], F32, tag="mm", name="ps_qkt")
                else:
                    ps = pool.tile([P, 512], F32, tag="rb", name="ps_fill")
                for k in range(KC):
                    nc.tensor.matmul(
                        ps[:, 0:512],
                        w_sb[:, k * 256 + m * P: k * 256 + (m + 1) * P],
                        xt_sb[:, k * T + nch * 512: k * T + (nch + 1) * 512],
                        start=(k == 0), stop=(k == KC - 1))
                nc.vector.tensor_copy(
                    dst[m][:, nch * 512:(nch + 1) * 512], ps[:, 0:512])

            def emit_v_group(mt):
                psv = psB.tile([P, 1024], F32, tag="mm", name="ps_v")
                for k in range(KC):
                    nc.tensor.matmul(
                        psv[:, 0:256],
                        xt_sb[:, k * T + mt * P: k * T + (mt + 1) * P],
                        wv_sb[:, k * 256:(k + 1) * 256],
                        start=(k == 0), stop=(k == KC - 1))
                for s in range(4):
                    nc.vector.tensor_scalar_mul(
                        vv_sb[:, mt, s, 0:D], psv[:, s * D:(s + 1) * D],
                        wcol_sb[:, mt, s: s + 1])

            # den columns for all (t, slot) in one strided copy (independent
            # of the V matmuls — disjoint byte ranges of vv)
            nc.vector.tensor_copy(vv_sb[:, :, :, 64], wcol_sb[:])

            # ---- phase 2: attention, flat software pipeline over strips.
            # Each strip = up to 2 key blocks of S^T for one (slot, q-chunk).
            # Issue order per step: S matmuls(i), exp/mask(i), PV(i-1) — the
            # PE queue is in-order, so PV lags one strip behind its exp.
            # Divides are deferred 2 strips past opsum completion so the
            # broadcast matmul never blocks the PE on a DVE dependency.
            # Interleave V groups with QT/KT groups upfront so each group's
            # DVE evictions overlap the other stream's matmuls.
            # m0 QKT groups + all V upfront; the m1 QKT groups (only needed
            # by slots 2/3, i.e. strip index >= 40) are injected as PE filler
            # into the ACT-bound slots-0/1 attention stream below.
            # KT m1 n3 is never read -> skipped.
            qkt_m0 = [0, 8, 1, 9, 2, 10, 3, 11]
            for i in range(16):
                if i < len(qkt_m0):
                    emit_qkt_group(qkt_m0[i])
                emit_v_group(i)
            fillers = [4, 12, 5, 13, 6, 14, 7]

            strips = []
            for s in range(4):
                for qc in range(QCH):
                    kmax = min(KBUD[s], 4 * qc + 4)
                    for g in range((kmax + 1) // 2):
                        kts = [kt for kt in (2 * g, 2 * g + 1) if kt < kmax]
                        strips.append((s, qc, g, kts, kmax))

            opsums = {}        # (s, qc) -> psum tile
            pending = None     # (strip, pst)
            divides = []       # [(emit_at_index, (s, qc))]

            def emit_pv(strip, pst):
                s, qc, g, kts, kmax = strip
                for d_, kt in enumerate(kts):
                    nc.tensor.matmul(
                        opsums[(s, qc)][:],
                        vv_sb[:, kt, s, :],
                        pst[:, d_ * 512:(d_ + 1) * 512],
                        start=(kt == 0), stop=(kt == kmax - 1))

            def emit_divide(s, qc):
                opsum = opsums.pop((s, qc))
                ot_t = ot_sb[s // 2]
                base = (s % 2) * D
                dh = rr.tile([1, 512], F16, tag="dh", name="dh")
                nc.vector.tensor_copy(dh[:], opsum[64:65, :])
                rb = psR.tile([D, 512], F32, tag="rb", name="rb")
                nc.tensor.matmul(rb[:], ones_sb[:], dh[:],
                                 start=True, stop=True)
                rbs = rbp.tile([D, 512], F32, tag="rbs", name="rbs")
                nc.vector.reciprocal_approx_fast(rbs[:], rb[:])
                nc.vector.tensor_mul(
                    ot_t[base:base + D, qc * 512:(qc + 1) * 512],
                    opsum[0:64, :], rbs[:])

            for i, strip in enumerate(strips):
                s, qc, g, kts, kmax = strip
                if fillers and 4 <= i and i % 5 == 4:
                    emit_qkt_group(fillers.pop(0), pool=psR)
                if (s, qc) not in opsums:
                    opsums[(s, qc)] = psO.tile([65, 512], F32, tag="o",
                                               name="opsum")
                qt_t = qt_sb[s // 2]
                kt_t = kt_sb[s // 2]
                base = (s % 2) * D
                w = len(kts)
                sps = psB.tile([P, 1024], F32, tag="mm", name="sps")
                for d_, kt in enumerate(kts):
                    nc.tensor.matmul(
                        sps[:, d_ * 512:(d_ + 1) * 512],
                        kt_t[base:base + D, kt * P:(kt + 1) * P],
                        qt_t[base:base + D, qc * 512:(qc + 1) * 512],
                        start=True, stop=True)
                pst = pp.tile([P, 1024], F16, tag="p", name="pst")
                nc.scalar.activation(pst[:, 0:512 * w], sps[:, 0:512 * w], ExpF)
                if g == 2 * qc:  # diagonal blocks delta 0,1
                    nc.vector.tensor_mul(pst[:, 0:512 * w], pst[:, 0:512 * w],
                                         masks_sb[:, 0:512 * w])
                elif g == 2 * qc + 1:  # diagonal blocks delta 2,3
                    nc.vector.tensor_mul(pst[:, 0:512 * w], pst[:, 0:512 * w],
                                         masks_sb[:, 1024:1024 + 512 * w])
                while divides and divides[0][0] <= i:
                    emit_divide(*divides.pop(0)[1])
                if pending is not None:
                    emit_pv(*pending)
                    ps_, qc_ = pending[0][0], pending[0][1]
                    if (s, qc) != (ps_, qc_):  # pending was last strip of its
                        divides.append((i + 2, (ps_, qc_)))  # (s,qc): divide
                pending = (strip, pst)
            emit_pv(*pending)
            divides.append((0, (pending[0][0], pending[0][1])))
            # Hoist the first two proj groups' j=0 matmuls (they read only
            # ot_sb[0], complete since slot 1) ahead of the serial divide
            # tail so the PE keeps working through it.
            early_ps = []
            for mt in range(2):
                ps = psB.tile([P, 1024], F32, tag="mm", name="ps_proj_e")
                for nch in range(2):
                    nc.tensor.matmul(
                        ps[:, nch * 512:(nch + 1) * 512],
                        ot_sb[0][:, mt * P:(mt + 1) * P],
                        wp_sb[:, nch * 512:(nch + 1) * 512],
                        start=True, stop=False)
                early_ps.append(ps)
            for _, key in divides:
                emit_divide(*key)

            # ---- phase 3: partial projection y = OT.T @ wp
            # [128,1024] psum per t-tile (4 matmuls); evictions alternate
            # between scalar and vector engines; output DMA split in two.
            for mt in range(NT):
                if mt < 2:
                    ps = early_ps[mt]
                    for nch in range(2):
                        nc.tensor.matmul(
                            ps[:, nch * 512:(nch + 1) * 512],
                            ot_sb[1][:, mt * P:(mt + 1) * P],
                            wp_sb[:, C + nch * 512: C + (nch + 1) * 512],
                            start=False, stop=True)
                else:
                    ps = psB.tile([P, 1024], F32, tag="mm", name="ps_proj")
                    for nch in range(2):
                        for j in range(2):
                            nc.tensor.matmul(
                                ps[:, nch * 512:(nch + 1) * 512],
                                ot_sb[j][:, mt * P:(mt + 1) * P],
                                wp_sb[:, j * C + nch * 512: j * C + (nch + 1) * 512],
                                start=(j == 0), stop=(j == 1))
                yt = yp.tile([P, 1024], F16, tag="y", name="yt")
                if mt % 2 == 0:
                    nc.scalar.copy(yt[:], ps[:])
                else:
                    nc.vector.tensor_copy(yt[:], ps[:])
                for h in range(2):
                    nc.sync.dma_start(
                        y_d[mt * P:(mt + 1) * P, h * 512:(h + 1) * 512],
                        yt[:, h * 512:(h + 1) * 512])

    nc.compile()
    return nc


def _host_prep(x, w_qkv, w_proj):
    """Per-core input maps."""
    slopes = _slopes()
    scale = 1.0 / np.sqrt(D)
    in_maps = []
    xt_by_b = [np.ascontiguousarray(x[b].T).astype(np.float16) for b in range(B)]

    # masks: delta in 0..3, [128, 512] each: valid iff r <= c - 128*delta
    rr_ = np.arange(P)[:, None]
    cc = np.arange(512)[None, :]
    masks = np.concatenate(
        [(rr_ <= cc - P * d).astype(np.float16) for d in range(4)], axis=1)

    group_data = []
    for g in range(4):
        H = GROUP_HEADS[g]
        cols = np.concatenate([np.arange(h * D, (h + 1) * D) for h in H])
        wq = (w_qkv[:, cols] * scale).astype(np.float16)
        wk = w_qkv[:, C + cols].astype(np.float16)
        wv = w_qkv[:, 2 * C + cols].astype(np.float16)
        wp = np.ascontiguousarray(w_proj[cols, :]).astype(np.float16)
        t = np.arange(T, dtype=np.float64)
        wcol = np.stack(
            [np.exp(-slopes[h] * t) for h in H], axis=1).astype(np.float32)
        group_data.append((wq, wk, wv, wp, wcol))

    for c in range(N_CORES):
        b, g = divmod(c, 4)
        wq, wk, wv, wp, wcol = group_data[g]
        in_maps.append({
            "xt": xt_by_b[b], "wq": wq, "wk": wk, "wv": wv, "wp": wp,
            "wcol": wcol, "masks": masks,
        })
    return in_maps


def kernel(x, w_qkv, w_proj):
    if "nc" not in _CACHE:
        _CACHE["nc"] = _build_program()
    nc = _CACHE["nc"]

    in_maps = _host_prep(np.asarray(x, np.float32), np.asarray(w_qkv, np.float32),
                         np.asarray(w_proj, np.float32))
    res = run_bass_kernel_spmd(nc, in_maps, list(range(N_CORES)), trace=TRACE)
    _CACHE["last_result"] = res

    y = np.zeros((B, T, C), dtype=np.float64)
    for c in range(N_CORES):
        b = c // 4
        y[b] += res.results[c]["y"].astype(np.float64)
    return y.astype(np.float32)



# revision 4
# speedup vs baseline: 1.2225x; 1.2225x over previous
"""Distributed Trainium2 kernel for EnhancedSelfAttention (causal attention
with additive ALiBi |i-j| bias) on 8 NeuronCores.

Math: for queries i and keys j<=i the bias is slope*(i-j), so
softmax_j(S_ij + slope*(i-j)) == softmax_j(S_ij - slope*j) — the slope*i term
is constant per row and cancels. Folding w_j = exp(-slope*j) into V's rows
(plus an appended w column for the denominator) turns the whole softmax into
exp(S) followed by a single PV matmul and a divide. w_j underflows to exactly
0 beyond j ~ 75/slope, so early heads only need the first few key blocks.

Sharding: 8 cores = 2 batches x 4 head groups (4 heads each). Each core
computes its partial projection output; partials are summed on the host.
All cores run one SPMD program: per-slot key-block budgets [16, 16, 10, 3]
with heads assigned to slots so that each head's needed blocks <= budget.

Attention works on S^T tiles ([key, query] layout) so the PV contraction
needs no transposes; exp runs on 1024-wide strips (2 key blocks) to amortize
ACT instruction overhead; the divide broadcasts den (fp16) via a ones-matmul
then applies reciprocal_approx_fast.
"""

import sys
import types

import numpy as np

import concourse.bass as bass
import concourse.mybir as mybir
import concourse.tile as tile
from concourse import bacc
from concourse.bass_utils import run_bass_kernel_spmd


def _ensure_axon_hooks():
    """concourse's trace path imports antenv.axon_hooks, which this image
    lacks; give it a no-op fallback so BASS_TRACE=1 can't crash the run."""
    try:
        import antenv.axon_hooks  # noqa: F401
    except Exception:
        try:
            import antenv
            mod = types.ModuleType("antenv.axon_hooks")
            mod.get_axon_ntff_profile_hook = lambda: None
            mod.set_axon_ntff_profile_hook = lambda h: None
            sys.modules["antenv.axon_hooks"] = mod
            antenv.axon_hooks = mod
        except Exception:
            pass


_ensure_axon_hooks()

F32 = mybir.dt.float32
F16 = mybir.dt.float16
ExpF = mybir.ActivationFunctionType.Exp

B, T, C = 2, 2048, 1024
NH, D = 16, 64
P = 128
NT = T // P            # 16 t tiles
KC = C // P            # 8 contraction subtiles for qkv/proj
QCH = 4                # q chunks of 512
KBUD = (16, 4, 1, 1)   # per-slot key-block budgets
N_CORES = 8

# Heads sorted by ALiBi slope into slots of equal budget: slot s of group g
# runs head SLOT_HEADS[s][g]; slot budgets are the max need of their heads
# (keys beyond a head's own cutoff carry w ~ 0 and cost nothing numerically).
GROUP_HEADS = [(15 - g, 11 - g, 7 - g, 3 - g) for g in range(4)]

TRACE = False  # test harness sets kernel.TRACE = True for NTFF profiling

_CACHE = {}


def _slopes():
    i = np.arange(1, NH + 1, dtype=np.float64)
    return (1.0 / np.power(2.0, 8.0 * i / NH)).astype(np.float64)


def _build_program():
    nc = bacc.Bacc("TRN2", target_bir_lowering=False, debug=False,
                   num_devices=N_CORES)

    xt_d = nc.dram_tensor("xt", [C, T], F16, kind="ExternalInput").ap()
    wq_d = nc.dram_tensor("wq", [C, 4 * D], F16, kind="ExternalInput").ap()
    wk_d = nc.dram_tensor("wk", [C, 4 * D], F16, kind="ExternalInput").ap()
    wv_d = nc.dram_tensor("wv", [C, 4 * D], F16, kind="ExternalInput").ap()
    wp_d = nc.dram_tensor("wp", [4 * D, C], F16, kind="ExternalInput").ap()
    wcol_d = nc.dram_tensor("wcol", [T, 4], F32, kind="ExternalInput").ap()
    masks_d = nc.dram_tensor("masks", [P, 4 * 512], F16, kind="ExternalInput").ap()
    y_d = nc.dram_tensor("y", [T, C], F16, kind="ExternalOutput").ap()

    with tile.TileContext(nc) as tc:
        with (
            nc.allow_low_precision(reason="fp16 matmul operands by design"),
            tc.tile_pool(name="const", bufs=1) as const,
            tc.tile_pool(name="psB", bufs=2, space="PSUM") as psB,
            tc.tile_pool(name="psO", bufs=3, space="PSUM") as psO,
            tc.tile_pool(name="psR", bufs=1, space="PSUM") as psR,
            tc.tile_pool(name="pp", bufs=4) as pp,
            tc.tile_pool(name="rr", bufs=3) as rr,
            tc.tile_pool(name="rbp", bufs=3) as rbp,
            tc.tile_pool(name="yp", bufs=4) as yp,
        ):
            # ---- persistent SBUF loads
            # Weights first (small), then xt in nch-major order: a QKV group
            # (m, nch) contracts over all 8 k-chunks but reads only its own
            # 512-column slice, so column-major arrival lets the first group
            # finish after ~2MB instead of the full 4MB.
            wq_sb = const.tile([P, KC * 4 * D], F16, tag="wq")
            wk_sb = const.tile([P, KC * 4 * D], F16, tag="wk")
            wv_sb = const.tile([P, KC * 4 * D], F16, tag="wv")
            for w_sb, w_d in ((wq_sb, wq_d), (wk_sb, wk_d), (wv_sb, wv_d)):
                for k in range(KC):
                    nc.sync.dma_start(w_sb[:, k * 256:(k + 1) * 256],
                                      w_d[k * P:(k + 1) * P, :])
            xt_sb = const.tile([P, KC * T], F16, tag="xt")       # 32KB/part
            # left halves of every k-chunk first: the first QKV groups
            # (nch 0/1) can start after 2MB arrives instead of 4MB
            for h in range(2):
                for k in range(KC):
                    nc.sync.dma_start(
                        xt_sb[:, k * T + h * 1024: k * T + (h + 1) * 1024],
                        xt_d[k * P:(k + 1) * P, h * 1024:(h + 1) * 1024])
            # DMA issue order follows first use: wcol feeds the first V
            # eviction (~27us), masks the first diagonal strip (~65us), wp
            # only the projection (~140us).
            wcol_sb = const.tile([P, NT, 4], F32, tag="wcol")
            nc.sync.dma_start(
                wcol_sb[:], wcol_d.rearrange("(n p) c -> p n c", p=P))
            masks_sb = const.tile([P, 4 * 512], F16, tag="masks")
            nc.sync.dma_start(masks_sb[:], masks_d[:])
            wp_sb = const.tile([P, 2 * C], F16, tag="wp")
            for j in range(2):
                nc.sync.dma_start(wp_sb[:, j * C:(j + 1) * C],
                                  wp_d[j * P:(j + 1) * P, :])
            ones_sb = const.tile([1, D], F16, tag="ones")
            nc.any.memset(ones_sb[:], 1.0)
            # warm the ACT exp table during the DMA wait
            warm_sb = const.tile([1, D], F16, tag="warm")
            nc.scalar.activation(warm_sb[:], ones_sb[:], ExpF)

            qt_sb = [const.tile([P, T], F16, tag=f"qt{m}", name=f"qt{m}")
                     for m in range(2)]
            kt_sb = [const.tile([P, T], F16, tag=f"kt{m}", name=f"kt{m}")
                     for m in range(2)]
            vv_sb = const.tile([P, NT, 4, 65], F16, tag="vv")
            ot_sb = [const.tile([P, T], F16, tag=f"ot{m}", name=f"ot{m}")
                     for m in range(2)]

            # ---- phase 1: QT/KT ([d, t] layout) interleaved with V groups so
            # each group's DVE evictions overlap the other stream's matmuls.
            def emit_qkt_group(i, pool=None):
                w_sb, dst = ((wq_sb, qt_sb), (wk_sb, kt_sb))[i // 8]
                m, nch = divmod(i % 8, QCH)
                if pool is None:
                    ps = psB.tile([P, 1024], F32, tag="mm", name="ps_qkt")
                else:
                    ps = pool.tile([P, 512], F32, tag="rb", name="ps_fill")
                for k in range(KC):
                    nc.tensor.matmul(
                        ps[:, 0:512],
                        w_sb[:, k * 256 + m * P: k * 256 + (m + 1) * P],
                        xt_sb[:, k * T + nch * 512: k * T + (nch + 1) * 512],
                        start=(k == 0), stop=(k == KC - 1))
                nc.vector.tensor_copy(
                    dst[m][:, nch * 512:(nch + 1) * 512], ps[:, 0:512])

            def emit_v_group(mt):
                # slots are budget-descending, so the slots needing key block
                # mt are a prefix: restrict matmul cols + evictions to them.
                nlive = sum(1 for s in range(4) if mt < KBUD[s])
                psv = psB.tile([P, 1024], F32, tag="mm", name="ps_v")
                for k in range(KC):
                    nc.tensor.matmul(
                        psv[:, 0:nlive * D],
                        xt_sb[:, k * T + mt * P: k * T + (mt + 1) * P],
                        wv_sb[:, k * 256:k * 256 + nlive * D],
                        start=(k == 0), stop=(k == KC - 1))
                for s in range(nlive):
                    nc.vector.tensor_scalar_mul(
                        vv_sb[:, mt, s, 0:D], psv[:, s * D:(s + 1) * D],
                        wcol_sb[:, mt, s: s + 1])

            # den columns for all (t, slot) in one strided copy (independent
            # of the V matmuls — disjoint byte ranges of vv)
            nc.vector.tensor_copy(vv_sb[:, :, :, 64], wcol_sb[:])

            # ---- phase 2: attention, flat software pipeline over strips.
            # Each strip = up to 2 key blocks of S^T for one (slot, q-chunk).
            # Issue order per step: S matmuls(i), exp/mask(i), PV(i-1) — the
            # PE queue is in-order, so PV lags one strip behind its exp.
            # Divides are deferred 2 strips past opsum completion so the
            # broadcast matmul never blocks the PE on a DVE dependency.
            # Interleave V groups with QT/KT groups upfront so each group's
            # DVE evictions overlap the other stream's matmuls.
            # m0 QKT groups + all V upfront; the m1 QKT groups (only needed
            # by slots 2/3, i.e. strip index >= 40) are injected as PE filler
            # into the ACT-bound slots-0/1 attention stream below.
            # KT m1 n3 is never read -> skipped.
            # KT m1 (slots 2/3) is only read for keys < 128*KBUD[2] -> just
            # nch0 (group 12); KT m1 nch1-3 are never read and skipped.
            qkt_m0 = [0, 8, 1, 9, 2, 10, 3, 11]
            for i in range(16):
                if i < len(qkt_m0):
                    emit_qkt_group(qkt_m0[i])
                emit_v_group(i)
            fillers = [4, 12, 5, 6, 7]

            strips = []
            for s in range(4):
                for qc in range(QCH):
                    kmax = min(KBUD[s], 4 * qc + 4)
                    for g in range((kmax + 1) // 2):
                        kts = [kt for kt in (2 * g, 2 * g + 1) if kt < kmax]
                        strips.append((s, qc, g, kts, kmax))

            opsums = {}        # (s, qc) -> psum tile
            pending = None     # (strip, pst)
            divides = []       # [(emit_at_index, (s, qc))]

            def emit_pv(strip, pst):
                s, qc, g, kts, kmax = strip
                for d_, kt in enumerate(kts):
                    nc.tensor.matmul(
                        opsums[(s, qc)][:],
                        vv_sb[:, kt, s, :],
                        pst[:, d_ * 512:(d_ + 1) * 512],
                        start=(kt == 0), stop=(kt == kmax - 1))

            def emit_divide(s, qc):
                opsum = opsums.pop((s, qc))
                ot_t = ot_sb[s // 2]
                base = (s % 2) * D
                dh = rr.tile([1, 512], F16, tag="dh", name="dh")
                nc.vector.tensor_copy(dh[:], opsum[64:65, :])
                rb = psR.tile([D, 512], F32, tag="rb", name="rb")
                nc.tensor.matmul(rb[:], ones_sb[:], dh[:],
                                 start=True, stop=True)
                rbs = rbp.tile([D, 512], F32, tag="rbs", name="rbs")
                nc.vector.reciprocal_approx_fast(rbs[:], rb[:])
                nc.vector.tensor_mul(
                    ot_t[base:base + D, qc * 512:(qc + 1) * 512],
                    opsum[0:64, :], rbs[:])

            for i, strip in enumerate(strips):
                s, qc, g, kts, kmax = strip
                if fillers and 4 <= i and i % 5 == 4:
                    emit_qkt_group(fillers.pop(0), pool=psR)
                if (s, qc) not in opsums:
                    opsums[(s, qc)] = psO.tile([65, 512], F32, tag="o",
                                               name="opsum")
                qt_t = qt_sb[s // 2]
                kt_t = kt_sb[s // 2]
                base = (s % 2) * D
                w = len(kts)
                sps = psB.tile([P, 1024], F32, tag="mm", name="sps")
                for d_, kt in enumerate(kts):
                    nc.tensor.matmul(
                        sps[:, d_ * 512:(d_ + 1) * 512],
                        kt_t[base:base + D, kt * P:(kt + 1) * P],
                        qt_t[base:base + D, qc * 512:(qc + 1) * 512],
                        start=True, stop=True)
                pst = pp.tile([P, 1024], F16, tag="p", name="pst")
                nc.scalar.activation(pst[:, 0:512 * w], sps[:, 0:512 * w], ExpF)
                if g == 2 * qc:  # diagonal blocks delta 0,1
                    nc.vector.tensor_mul(pst[:, 0:512 * w], pst[:, 0:512 * w],
                                         masks_sb[:, 0:512 * w])
                elif g == 2 * qc + 1:  # diagonal blocks delta 2,3
                    nc.vector.tensor_mul(pst[:, 0:512 * w], pst[:, 0:512 * w],
                                         masks_sb[:, 1024:1024 + 512 * w])
                while divides and divides[0][0] <= i:
                    emit_divide(*divides.pop(0)[1])
                if pending is not None:
                    emit_pv(*pending)
                    ps_, qc_ = pending[0][0], pending[0][1]
                    if (s, qc) != (ps_, qc_):  # pending was last strip of its
                        divides.append((i + 2, (ps_, qc_)))  # (s,qc): divide
                pending = (strip, pst)
            emit_pv(*pending)
            divides.append((0, (pending[0][0], pending[0][1])))
            # Hoist the first two proj groups' j=0 matmuls (they read only
            # ot_sb[0], complete since slot 1) ahead of the serial divide
            # tail so the PE keeps working through it.
            early_ps = []
            for mt in range(2):
                ps = psB.tile([P, 1024], F32, tag="mm", name="ps_proj_e")
                for nch in range(2):
                    nc.tensor.matmul(
                        ps[:, nch * 512:(nch + 1) * 512],
                        ot_sb[0][:, mt * P:(mt + 1) * P],
                        wp_sb[:, nch * 512:(nch + 1) * 512],
                        start=True, stop=False)
                early_ps.append(ps)
            for _, key in divides:
                emit_divide(*key)

            # ---- phase 3: partial projection y = OT.T @ wp
            # [128,1024] psum per t-tile (4 matmuls); evictions alternate
            # between scalar and vector engines; output DMA split in two.
            for mt in range(NT):
                if mt < 2:
                    ps = early_ps[mt]
                    for nch in range(2):
                        nc.tensor.matmul(
                            ps[:, nch * 512:(nch + 1) * 512],
                            ot_sb[1][:, mt * P:(mt + 1) * P],
                            wp_sb[:, C + nch * 512: C + (nch + 1) * 512],
                            start=False, stop=True)
                else:
                    ps = psB.tile([P, 1024], F32, tag="mm", name="ps_proj")
                    for nch in range(2):
                        for j in range(2):
                            nc.tensor.matmul(
                                ps[:, nch * 512:(nch + 1) * 512],
                                ot_sb[j][:, mt * P:(mt + 1) * P],
                                wp_sb[:, j * C + nch * 512: j * C + (nch + 1) * 512],
                                start=(j == 0), stop=(j == 1))
                yt = yp.tile([P, 1024], F16, tag="y", name="yt")
                if mt % 2 == 0:
                    nc.scalar.copy(yt[:], ps[:])
                else:
                    nc.vector.tensor_copy(yt[:], ps[:])
                for h in range(2):
                    nc.sync.dma_start(
                        y_d[mt * P:(mt + 1) * P, h * 512:(h + 1) * 512],
                        yt[:, h * 512:(h + 1) * 512])

    nc.compile()
    return nc


def _host_prep(x, w_qkv, w_proj):
    """Per-core input maps."""
    slopes = _slopes()
    scale = 1.0 / np.sqrt(D)
    in_maps = []
    xt_by_b = [np.ascontiguousarray(x[b].T).astype(np.float16) for b in range(B)]

    # masks: delta in 0..3, [128, 512] each: valid iff r <= c - 128*delta
    rr_ = np.arange(P)[:, None]
    cc = np.arange(512)[None, :]
    masks = np.concatenate(
        [(rr_ <= cc - P * d).astype(np.float16) for d in range(4)], axis=1)

    group_data = []
    for g in range(4):
        H = GROUP_HEADS[g]
        cols = np.concatenate([np.arange(h * D, (h + 1) * D) for h in H])
        wq = (w_qkv[:, cols] * scale).astype(np.float16)
        wk = w_qkv[:, C + cols].astype(np.float16)
        wv = w_qkv[:, 2 * C + cols].astype(np.float16)
        wp = np.ascontiguousarray(w_proj[cols, :]).astype(np.float16)
        t = np.arange(T, dtype=np.float64)
        wcol = np.stack(
            [np.exp(-slopes[h] * t) for h in H], axis=1).astype(np.float32)
        group_data.append((wq, wk, wv, wp, wcol))

    for c in range(N_CORES):
        b, g = divmod(c, 4)
        wq, wk, wv, wp, wcol = group_data[g]
        in_maps.append({
            "xt": xt_by_b[b], "wq": wq, "wk": wk, "wv": wv, "wp": wp,
            "wcol": wcol, "masks": masks,
        })
    return in_maps


def kernel(x, w_qkv, w_proj):
    if "nc" not in _CACHE:
        _CACHE["nc"] = _build_program()
    nc = _CACHE["nc"]

    in_maps = _host_prep(np.asarray(x, np.float32), np.asarray(w_qkv, np.float32),
                         np.asarray(w_proj, np.float32))
    res = run_bass_kernel_spmd(nc, in_maps, list(range(N_CORES)), trace=TRACE)
    _CACHE["last_result"] = res

    y = np.zeros((B, T, C), dtype=np.float64)
    for c in range(N_CORES):
        b = c // 4
        y[b] += res.results[c]["y"].astype(np.float64)
    return y.astype(np.float32)



# revision 20
# speedup vs baseline: 1.3120x; 1.0732x over previous
"""Distributed Trainium2 kernel for EnhancedSelfAttention (causal attention
with additive ALiBi |i-j| bias) on 8 NeuronCores.

Math: for keys j<=i the bias slope*(i-j) reduces (after the per-row constant
cancels in softmax) to weights w_j = exp(-m*j).  Split w_j = blk * inb with
blk = exp(-m*128*(j//128)) folded into V's key-block rows (and the appended
den column), and inb = exp(-m*(j%128)) folded into the exp as a per-partition
ACT bias.  P' = exp(S/8192 - m*p) then feeds a single PV matmul per block
pair plus a divide; blk scaling makes far blocks underflow to exactly the
truncation the per-slot budgets assume.

Precision: Q/K are produced from fp8(e4m3) x and 32x-prescaled fp8 weights
via DoubleRow matmuls (2 contraction chunks per instruction), attention
P'/V run in fp8 with DoubleRow PV for the far-context slots (0,1) on query
chunks >= 1, and in fp16 for query chunk 0 / fast-decay slots where few keys
get no averaging dilution.  Projection stays fp16.  Measured end-to-end
max-rel error ~1.2e-2 vs the 2e-2 gate.

Sharding: 8 cores = 2 batches x 4 head groups; heads sorted by slope into 4
slots of uniform budget KBUD=(16,4,1,1) key blocks; partial projection
outputs summed on the host.
"""

import sys
import types

import numpy as np
import ml_dtypes

import concourse.bass as bass
import concourse.mybir as mybir
import concourse.tile as tile
from concourse import bacc
from concourse.bass_utils import run_bass_kernel_spmd


def _ensure_axon_hooks():
    try:
        import antenv.axon_hooks  # noqa: F401
    except Exception:
        try:
            import antenv
            mod = types.ModuleType("antenv.axon_hooks")
            mod.get_axon_ntff_profile_hook = lambda: None
            mod.set_axon_ntff_profile_hook = lambda h: None
            sys.modules["antenv.axon_hooks"] = mod
            antenv.axon_hooks = mod
        except Exception:
            pass


_ensure_axon_hooks()

F32 = mybir.dt.float32
F16 = mybir.dt.float16
F8 = mybir.dt.float8e4
DR = mybir.MatmulPerfMode.DoubleRow
ExpF = mybir.ActivationFunctionType.Exp
E4NP = ml_dtypes.float8_e4m3

B, T, C = 2, 2048, 1024
NH, D = 16, 64
P = 128
NT = T // P            # 16 key/t blocks
KC = C // P            # 8 contraction chunks
QCH = 4                # q chunks of 512
KBUD = (16, 4, 1, 1)   # per-slot key-block budgets
N_CORES = 8

# slot s of group g runs head SLOT_HEADS[s][g] (sorted by ALiBi slope so a
# slot's budget is the max need of its heads).
SLOT_HEADS = [[15 - g for g in range(4)], [11 - g for g in range(4)],
              [7 - g for g in range(4)], [3 - g for g in range(4)]]

TRACE = False

_CACHE = {}


def _slopes():
    i = np.arange(1, NH + 1, dtype=np.float64)
    return 1.0 / np.power(2.0, 8.0 * i / NH)


def _build_program():
    nc = bacc.Bacc("TRN2", target_bir_lowering=False, debug=False,
                   num_devices=N_CORES)

    xt8_d = nc.dram_tensor("xt8", [C, T], F8, kind="ExternalInput").ap()
    xtf_d = nc.dram_tensor("xtf", [C, 512], F16, kind="ExternalInput").ap()
    wq8_d = nc.dram_tensor("wq8", [C, 4 * D], F8, kind="ExternalInput").ap()
    wk8_d = nc.dram_tensor("wk8", [C, 4 * D], F8, kind="ExternalInput").ap()
    wv_d = nc.dram_tensor("wv", [C, 4 * D], F16, kind="ExternalInput").ap()
    wp_d = nc.dram_tensor("wp", [4 * D, C], F16, kind="ExternalInput").ap()
    masks_d = nc.dram_tensor("masks", [P, 4 * 512], F8, kind="ExternalInput").ap()
    bias_d = nc.dram_tensor("bias", [P, 4], F32, kind="ExternalInput").ap()
    vsc_d = nc.dram_tensor("vsc", [P, NT * 4], F32, kind="ExternalInput").ap()
    y_d = nc.dram_tensor("y", [T, C], F16, kind="ExternalOutput").ap()

    with tile.TileContext(nc) as tc:
        with (
            nc.allow_low_precision(reason="fp8/fp16 matmul operands by design"),
            tc.tile_pool(name="const", bufs=1) as const,
            tc.tile_pool(name="psA", bufs=2, space="PSUM") as psA,
            tc.tile_pool(name="psO", bufs=3, space="PSUM") as psO,
            tc.tile_pool(name="psR", bufs=1, space="PSUM") as psR,
            tc.tile_pool(name="pp", bufs=4) as pp,
            tc.tile_pool(name="rr", bufs=3) as rr,
            tc.tile_pool(name="rbp", bufs=2) as rbp,
            tc.tile_pool(name="yp", bufs=4) as yp,
        ):
            # ---- persistent SBUF loads (order = first use)
            wq8_sb = const.tile([P, KC, 4 * D], F8, tag="wq8")
            wk8_sb = const.tile([P, KC, 4 * D], F8, tag="wk8")
            nc.sync.dma_start(wq8_sb[:], wq8_d.rearrange("(k p) n -> p k n", p=P))
            nc.sync.dma_start(wk8_sb[:], wk8_d.rearrange("(k p) n -> p k n", p=P))
            xt8_sb = const.tile([P, KC, T], F8, tag="xt8")
            xt8_r = xt8_d.rearrange("(k p) t -> p k t", p=P)
            for n in range(QCH):
                nc.sync.dma_start(xt8_sb[:, :, n * 512:(n + 1) * 512],
                                  xt8_r[:, :, n * 512:(n + 1) * 512])
            wv_sb = const.tile([P, KC, 4 * D], F16, tag="wv")
            nc.sync.dma_start(wv_sb[:], wv_d.rearrange("(k p) n -> p k n", p=P))
            xtf_sb = const.tile([P, KC, 512], F16, tag="xtf")
            nc.sync.dma_start(xtf_sb[:], xtf_d.rearrange("(k p) t -> p k t", p=P))
            masks_sb = const.tile([P, 4 * 512], F8, tag="masks")
            nc.sync.dma_start(masks_sb[:], masks_d[:])
            bias_sb = const.tile([P, 4], F32, tag="bias")
            nc.sync.dma_start(bias_sb[:], bias_d[:])
            vsc_sb = const.tile([P, NT, 4], F32, tag="vsc")
            nc.sync.dma_start(vsc_sb[:], vsc_d.rearrange("p (n s) -> p n s", s=4))
            wp_sb = const.tile([P, 2, C], F16, tag="wp")
            nc.sync.dma_start(wp_sb[:], wp_d.rearrange("(j p) c -> p j c", p=P))

            ones_sb = const.tile([1, D], F16, tag="ones")
            nc.any.memset(ones_sb[:], 1.0)
            # warm the ACT exp table + the PE HAM clock gate during DMA wait
            warm_sb = const.tile([1, D], F16, tag="warm")
            nc.scalar.activation(warm_sb[:], ones_sb[:], ExpF)
            wps = psA.tile([P, 1024], F32, tag="mm", name="ps_warm")
            for i in range(24):
                nc.tensor.matmul(wps[:, 0:256], wq8_sb[:, 0, 0:P],
                                 wq8_sb[:, 0, :], start=True, stop=True)

            qt_sb = [const.tile([P, T], F16, tag=f"qt{m}", name=f"qt{m}")
                     for m in range(2)]
            kt0_sb = const.tile([P, T], F16, tag="kt0")
            kt1_sb = const.tile([P, P], F16, tag="kt1")
            vv8_sb = const.tile([P, NT, 2, 72], F8, tag="vv8")
            vv16_sb = const.tile([P, 4, 4, 66], F16, tag="vv16")
            ot_sb = const.tile([P, 2, T], F16, tag="ot")

            # den columns = per-block factor exp(-m*128*kt)
            nc.vector.tensor_copy(vv8_sb[:, :, :, 64], vsc_sb[:, :, 0:2])
            nc.vector.tensor_copy(vv16_sb[:, :, :, 64], vsc_sb[:, 0:4, :])

            # ---- phase 1: Q^T/K^T via fp8 DoubleRow (2 contraction chunks
            # per matmul), V via fp16 (blocks 0-3 from fp16 x, 4-15 from fp8
            # x stationary with fp16 wv moving, slot0 columns only).
            def emit_qkt(which, m, nch, width=512):
                w_sb, dst = ((wq8_sb, qt_sb[m]),
                             (wk8_sb, kt0_sb if m == 0 else kt1_sb))[which]
                ps = psA.tile([P, 1024], F32, tag="mm", name="ps_qkt")
                for kp in range(KC // 2):
                    nc.tensor.matmul(
                        ps[:, 0:width],
                        w_sb[:, 2 * kp:2 * kp + 2, m * P:(m + 1) * P],
                        xt8_sb[:, 2 * kp:2 * kp + 2,
                               nch * 512:nch * 512 + width],
                        start=(kp == 0), stop=(kp == KC // 2 - 1),
                        perf_mode=DR)
                if which == 1 and m == 1:
                    nc.vector.tensor_copy(dst[:, 0:width], ps[:, 0:width])
                else:
                    nc.vector.tensor_copy(
                        dst[:, nch * 512:nch * 512 + width], ps[:, 0:width])

            def emit_v_near(mt):
                nlive = sum(1 for s in range(4) if mt < KBUD[s])
                psv = psA.tile([P, 1024], F32, tag="mm", name="ps_vn")
                for k in range(KC):
                    nc.tensor.matmul(
                        psv[:, 0:nlive * D],
                        xtf_sb[:, k, mt * P:(mt + 1) * P],
                        wv_sb[:, k, 0:nlive * D],
                        start=(k == 0), stop=(k == KC - 1))
                for s in range(nlive):
                    nc.vector.tensor_scalar_mul(
                        vv16_sb[:, mt, s, 0:D], psv[:, s * D:(s + 1) * D],
                        vsc_sb[:, mt, s:s + 1])
                for s in range(min(nlive, 2)):
                    nc.vector.tensor_scalar_mul(
                        vv8_sb[:, mt, s, 0:D], psv[:, s * D:(s + 1) * D],
                        vsc_sb[:, mt, s:s + 1])

            def emit_v_far(mt):
                psv = psA.tile([P, 1024], F32, tag="mm", name="ps_vf")
                for k in range(KC):
                    nc.tensor.matmul(
                        psv[:, 0:D],
                        xt8_sb[:, k, mt * P:(mt + 1) * P],
                        wv_sb[:, k, 0:D],
                        start=(k == 0), stop=(k == KC - 1))
                nc.vector.tensor_scalar_mul(
                    vv8_sb[:, mt, 0, 0:D], psv[:, 0:D], vsc_sb[:, mt, 0:1])

            for nch in range(QCH):
                emit_qkt(0, 0, nch)
                emit_qkt(0, 1, nch)
                emit_qkt(1, 0, nch)
                if nch == 0:
                    emit_qkt(1, 1, 0, width=P)
            for mt in range(4):
                emit_v_near(mt)
            for mt in range(4, NT):
                emit_v_far(mt)

            # ---- phase 2: attention (qc-major so projection of finished
            # query chunks overlaps later chunks) with the classic pipeline:
            # S(i) matmuls, exp(i), PV(i-1); paired divides; proj interleave.
            strips = []
            for qc in range(QCH):
                for s in range(4):
                    kmax = min(KBUD[s], 4 * qc + 4)
                    for g in range((kmax + 1) // 2):
                        kts = [kt for kt in (2 * g, 2 * g + 1) if kt < kmax]
                        strips.append((s, qc, g, kts, kmax))

            opsums = {}
            pending = None
            actions = []   # deferred (emit_at, fn) actions
            proj_done = [False] * NT

            def emit_pv(strip, pst):
                s, qc, g, kts, kmax = strip
                fp8 = (s <= 1 and qc >= 1)
                op = opsums[(s, qc)]
                if fp8:
                    nc.tensor.matmul(
                        op[:], vv8_sb[:, 2 * g:2 * g + 2, s, 0:65],
                        pst[:, 0:2, :],
                        start=(g == 0), stop=(2 * g + 2 >= kmax),
                        perf_mode=DR)
                else:
                    for d_, kt in enumerate(kts):
                        nc.tensor.matmul(
                            op[:], vv16_sb[:, kt, s, 0:65], pst[:, d_, :],
                            start=(kt == 0), stop=(kt == kmax - 1))

            def emit_divide(sa, qc):
                for s_ in (sa, sa + 1):
                    op = opsums.pop((s_, qc))
                    dh = rr.tile([1, 512], F16, tag="dh", name="dh")
                    nc.vector.tensor_copy(dh[:], op[64:65, :])
                    rb = psR.tile([D, 512], F32, tag="rb", name="rb")
                    nc.tensor.matmul(rb[:], ones_sb[:], dh[:],
                                     start=True, stop=True)
                    rbs = rbp.tile([D, 512], F32, tag="rbs", name="rbs")
                    nc.vector.reciprocal_approx_fast(rbs[:], rb[:])
                    base = (s_ % 2) * D
                    nc.vector.tensor_mul(
                        ot_sb[base:base + D, sa // 2, qc * 512:(qc + 1) * 512],
                        op[0:64, :], rbs[:])

            def emit_proj(mt):
                ps = psA.tile([P, 1024], F32, tag="mm", name="ps_proj")
                for nch2 in range(2):
                    for j in range(2):
                        nc.tensor.matmul(
                            ps[:, nch2 * 512:(nch2 + 1) * 512],
                            ot_sb[:, j, mt * P:(mt + 1) * P],
                            wp_sb[:, j, nch2 * 512:(nch2 + 1) * 512],
                            start=(j == 0), stop=(j == 1))
                yt = yp.tile([P, 1024], F16, tag="y", name="yt")
                if mt % 2 == 0:
                    nc.scalar.copy(yt[:], ps[:])
                else:
                    nc.vector.tensor_copy(yt[:], ps[:])
                for h in range(2):
                    nc.sync.dma_start(
                        y_d[mt * P:(mt + 1) * P, h * 512:(h + 1) * 512],
                        yt[:, h * 512:(h + 1) * 512])
                proj_done[mt] = True

            for i, strip in enumerate(strips):
                s, qc, g, kts, kmax = strip
                fp8 = (s <= 1 and qc >= 1)
                if (s, qc) not in opsums:
                    opsums[(s, qc)] = psO.tile([65, 512], F32, tag="o",
                                               name="opsum")
                if s < 2:
                    qt_t, kt_t = qt_sb[0], kt0_sb
                else:
                    qt_t, kt_t = qt_sb[1], kt1_sb
                base = (s % 2) * D
                w = len(kts)
                sps = psA.tile([P, 1024], F32, tag="mm", name="sps")
                for d_, kt in enumerate(kts):
                    nc.tensor.matmul(
                        sps[:, d_ * 512:(d_ + 1) * 512],
                        kt_t[base:base + D, kt * P:(kt + 1) * P],
                        qt_t[base:base + D, qc * 512:(qc + 1) * 512],
                        start=True, stop=True)
                pst = pp.tile([P, 2, 512], F8 if fp8 else F16,
                              tag="p8" if fp8 else "p16", name="pst")
                nc.scalar.activation(pst[:, 0:w, :], sps[:, 0:512 * w].rearrange(
                    "p (w n) -> p w n", n=512),
                    ExpF, bias=bias_sb[:, s:s + 1], scale=1.0 / 8192.0)
                if g == 2 * qc:
                    nc.vector.tensor_mul(
                        pst[:, 0:w, :],
                        pst[:, 0:w, :],
                        masks_sb[:, 0:512 * w].rearrange("p (w n) -> p w n", n=512))
                elif g == 2 * qc + 1:
                    nc.vector.tensor_mul(
                        pst[:, 0:w, :],
                        pst[:, 0:w, :],
                        masks_sb[:, 1024:1024 + 512 * w].rearrange(
                            "p (w n) -> p w n", n=512))
                while actions and actions[0][0] <= i:
                    actions.pop(0)[1]()
                if pending is not None:
                    emit_pv(*pending)
                    ps_, qc_ = pending[0][0], pending[0][1]
                    if (s, qc) != (ps_, qc_) and ps_ % 2 == 1:
                        actions.append((i + 1, (lambda a=ps_ - 1, b=qc_:
                                                emit_divide(a, b))))
                        if ps_ == 3:
                            mts = [4 * qc_, 4 * qc_ + 1, 4 * qc_ + 2, 4 * qc_ + 3]
                            for off, mt in enumerate(mts):
                                actions.append((i + 2 + 2 * off,
                                                (lambda m=mt: emit_proj(m))))
                pending = (strip, pst)
            emit_pv(*pending)
            for _, fn in actions:  # flush (incl. divide(0,3) if unfired)
                fn()
            emit_divide(2, 3)
            for mt in range(NT):
                if not proj_done[mt]:
                    emit_proj(mt)

    nc.compile()
    return nc


def _host_prep(x, w_qkv, w_proj):
    slopes = _slopes()
    in_maps = []
    xt_by_b = [np.ascontiguousarray(x[b].T) for b in range(B)]

    rr_ = np.arange(P)[:, None]
    cc = np.arange(512)[None, :]
    masks = np.concatenate(
        [(rr_ <= cc - P * d).astype(E4NP) for d in range(4)], axis=1)


    group_data = []
    for g in range(4):
        H = [SLOT_HEADS[s][g] for s in range(4)]
        cols = np.concatenate([np.arange(h * D, (h + 1) * D) for h in H])
        wq8 = (32.0 * w_qkv[:, cols]).astype(E4NP)
        wk8 = (32.0 * w_qkv[:, C + cols]).astype(E4NP)
        wv = w_qkv[:, 2 * C + cols].astype(np.float16)
        wp = np.ascontiguousarray(w_proj[cols, :]).astype(np.float16)
        bias = np.stack(
            [-slopes[h] * np.arange(P, dtype=np.float64) for h in H],
            axis=1).astype(np.float32)
        vsc = np.broadcast_to(
            np.exp(-np.outer(128.0 * np.arange(NT),
                             np.array([slopes[h] for h in H]))
                   ).astype(np.float32).reshape(1, NT * 4),
            (P, NT * 4)).copy()
        group_data.append((wq8, wk8, wv, wp, bias, vsc))

    for c in range(N_CORES):
        b, g = divmod(c, 4)
        wq8, wk8, wv, wp, bias, vsc = group_data[g]
        xt = xt_by_b[b]
        in_maps.append({
            "xt8": np.clip(xt, -240, 240).astype(E4NP),
            "xtf": xt[:, 0:512].astype(np.float16),
            "wq8": wq8, "wk8": wk8, "wv": wv, "wp": wp,
            "masks": masks, "bias": bias, "vsc": vsc,
        })
    return in_maps


def kernel(x, w_qkv, w_proj):
    if "nc" not in _CACHE:
        _CACHE["nc"] = _build_program()
    nc = _CACHE["nc"]

    in_maps = _host_prep(np.asarray(x, np.float32), np.asarray(w_qkv, np.float32),
                         np.asarray(w_proj, np.float32))
    res = run_bass_kernel_spmd(nc, in_maps, list(range(N_CORES)), trace=TRACE)
    _CACHE["last_result"] = res

    y = np.zeros((B, T, C), dtype=np.float64)
    for c in range(N_CORES):
        b = c // 4
        y[b] += res.results[c]["y"].astype(np.float64)
    return y.astype(np.float32)


# revision 29
# speedup vs baseline: 1.5193x; 1.1580x over previous
"""Distributed Trainium2 kernel for EnhancedSelfAttention (causal attention
with additive ALiBi |i-j| bias) on 8 NeuronCores.

Math: for keys j<=i the bias slope*(i-j) reduces (after the per-row constant
cancels in softmax) to weights w_j = exp(-m*j).  Split w_j = blk * inb with
blk = exp(-m*128*(j//128)) folded into V's key-block rows (and the appended
den column), and inb = exp(-m*(j%128)) folded into the exp as a per-partition
ACT bias.  P' = exp(S/8192 - m*p) then feeds a single PV matmul per block
pair plus a divide; blk scaling makes far blocks underflow to exactly the
truncation the per-slot budgets assume.

Precision: Q/K are produced from fp8(e4m3) x and 32x-prescaled fp8 weights
via DoubleRow matmuls (2 contraction chunks per instruction), attention
P'/V run in fp8 with DoubleRow PV for the far-context slots (0,1) on query
chunks >= 1, and in fp16 for query chunk 0 / fast-decay slots where few keys
get no averaging dilution.  Projection stays fp16.  Measured end-to-end
max-rel error ~1.2e-2 vs the 2e-2 gate.

Sharding: 8 cores = 2 batches x 4 head groups; heads sorted by slope into 4
slots of uniform budget KBUD=(16,4,1,1) key blocks; partial projection
outputs summed on the host.
"""

import sys
import types

import numpy as np
import ml_dtypes

import concourse.bass as bass
import concourse.mybir as mybir
import concourse.tile as tile
from concourse import bacc
from concourse.bass_utils import run_bass_kernel_spmd


def _ensure_axon_hooks():
    try:
        import antenv.axon_hooks  # noqa: F401
    except Exception:
        try:
            import antenv
            mod = types.ModuleType("antenv.axon_hooks")
            mod.get_axon_ntff_profile_hook = lambda: None
            mod.set_axon_ntff_profile_hook = lambda h: None
            sys.modules["antenv.axon_hooks"] = mod
            antenv.axon_hooks = mod
        except Exception:
            pass


_ensure_axon_hooks()

F32 = mybir.dt.float32
F16 = mybir.dt.float16
F8 = mybir.dt.float8e4
DR = mybir.MatmulPerfMode.DoubleRow
ExpF = mybir.ActivationFunctionType.Exp
E4NP = ml_dtypes.float8_e4m3

B, T, C = 2, 2048, 1024
NH, D = 16, 64
P = 128
NT = T // P            # 16 key/t blocks
KC = C // P            # 8 contraction chunks
QCH = 4                # q chunks of 512
KBUD = (16, 4, 1, 1)   # per-slot key-block budgets
N_CORES = 8

# slot s of group g runs head SLOT_HEADS[s][g] (sorted by ALiBi slope so a
# slot's budget is the max need of its heads).
SLOT_HEADS = [[15 - g for g in range(4)], [11 - g for g in range(4)],
              [7 - g for g in range(4)], [3 - g for g in range(4)]]

TRACE = False

_CACHE = {}


def _slopes():
    i = np.arange(1, NH + 1, dtype=np.float64)
    return 1.0 / np.power(2.0, 8.0 * i / NH)


def _build_program():
    nc = bacc.Bacc("TRN2", target_bir_lowering=False, debug=False,
                   num_devices=N_CORES)

    # All inputs are host-pre-arranged to [128, ...] partition-major layouts
    # so every DMA moves long contiguous runs per partition row.
    xt8_d = nc.dram_tensor("xt8", [P, QCH * KC * 512], F8,
                           kind="ExternalInput").ap()
    xtf_d = nc.dram_tensor("xtf", [P, KC * 512], F16, kind="ExternalInput").ap()
    wq8_d = nc.dram_tensor("wq8", [P, KC * 4 * D], F8, kind="ExternalInput").ap()
    wk8_d = nc.dram_tensor("wk8", [P, KC * 4 * D], F8, kind="ExternalInput").ap()
    wv_d = nc.dram_tensor("wv", [P, KC * 4 * D], F16, kind="ExternalInput").ap()
    wp_d = nc.dram_tensor("wp", [P, 2 * C], F16, kind="ExternalInput").ap()
    masks_d = nc.dram_tensor("masks", [P, 4 * 512], F8, kind="ExternalInput").ap()
    bias_d = nc.dram_tensor("bias", [P, 4], F32, kind="ExternalInput").ap()
    vsc_d = nc.dram_tensor("vsc", [P, NT * 4], F32, kind="ExternalInput").ap()
    y_d = nc.dram_tensor("y", [T, C], F16, kind="ExternalOutput").ap()

    with tile.TileContext(nc) as tc:
        with (
            nc.allow_low_precision(reason="fp8/fp16 matmul operands by design"),
            tc.tile_pool(name="const", bufs=1) as const,
            tc.tile_pool(name="psA", bufs=2, space="PSUM") as psA,
            tc.tile_pool(name="psO", bufs=3, space="PSUM") as psO,
            tc.tile_pool(name="psR", bufs=1, space="PSUM") as psR,
            tc.tile_pool(name="pp", bufs=4) as pp,
            tc.tile_pool(name="rr", bufs=3) as rr,
            tc.tile_pool(name="rbp", bufs=2) as rbp,
            tc.tile_pool(name="yp", bufs=4) as yp,
        ):
            # ---- persistent SBUF loads (order = first use)
            wq8_sb = const.tile([P, KC, 4 * D], F8, tag="wq8")
            wk8_sb = const.tile([P, KC, 4 * D], F8, tag="wk8")
            nc.sync.dma_start(wq8_sb[:], wq8_d.rearrange("p (k n) -> p k n", k=KC))
            nc.sync.dma_start(wk8_sb[:], wk8_d.rearrange("p (k n) -> p k n", k=KC))
            xt8_sb = const.tile([P, QCH, KC, 512], F8, tag="xt8")
            xt8_r = xt8_d.rearrange("p (n k t) -> p n k t", n=QCH, k=KC)
            for n in range(QCH):
                nc.sync.dma_start(xt8_sb[:, n], xt8_r[:, n])
            wv_sb = const.tile([P, KC, 4 * D], F16, tag="wv")
            nc.sync.dma_start(wv_sb[:], wv_d.rearrange("p (k n) -> p k n", k=KC))
            xtf_sb = const.tile([P, KC, 512], F16, tag="xtf")
            nc.sync.dma_start(xtf_sb[:], xtf_d.rearrange("p (k t) -> p k t", k=KC))
            masks_sb = const.tile([P, 4 * 512], F8, tag="masks")
            nc.sync.dma_start(masks_sb[:], masks_d[:])
            bias_sb = const.tile([P, 4], F32, tag="bias")
            nc.sync.dma_start(bias_sb[:], bias_d[:])
            vsc_sb = const.tile([P, NT, 4], F32, tag="vsc")
            nc.sync.dma_start(vsc_sb[:], vsc_d.rearrange("p (n s) -> p n s", s=4))
            wp_sb = const.tile([P, 2, C], F16, tag="wp")
            nc.sync.dma_start(wp_sb[:], wp_d.rearrange("p (j c) -> p j c", j=2))

            ones_sb = const.tile([1, D], F16, tag="ones")
            nc.any.memset(ones_sb[:], 1.0)
            # warm the ACT exp table + the PE HAM clock gate during DMA wait
            warm_sb = const.tile([1, D], F16, tag="warm")
            nc.scalar.activation(warm_sb[:], ones_sb[:], ExpF)
            wps = psA.tile([P, 1024], F32, tag="mm", name="ps_warm")
            for i in range(24):
                nc.tensor.matmul(wps[:, 0:256], wq8_sb[:, 0, 0:P],
                                 wq8_sb[:, 0, :], start=True, stop=True)

            qt_sb = [const.tile([P, T], F16, tag=f"qt{m}", name=f"qt{m}")
                     for m in range(2)]
            kt0_sb = const.tile([P, T], F16, tag="kt0")
            kt1_sb = const.tile([P, P], F16, tag="kt1")
            vv8_sb = const.tile([P, NT, 2, 72], F8, tag="vv8")
            vv16_sb = const.tile([P, 4, 4, 66], F16, tag="vv16")
            ot_sb = const.tile([P, 2, T], F16, tag="ot")

            # den columns = per-block factor exp(-m*128*kt)
            nc.vector.tensor_copy(vv8_sb[:, :, :, 64], vsc_sb[:, :, 0:2])
            nc.vector.tensor_copy(vv16_sb[:, :, :, 64], vsc_sb[:, 0:4, :])

            # ---- phase 1: Q^T/K^T via fp8 DoubleRow (2 contraction chunks
            # per matmul), V via fp16 (blocks 0-3 from fp16 x, 4-15 from fp8
            # x stationary with fp16 wv moving, slot0 columns only).
            def emit_qkt(which, m, nch, width=512):
                w_sb, dst = ((wq8_sb, qt_sb[m]),
                             (wk8_sb, kt0_sb if m == 0 else kt1_sb))[which]
                ps = psA.tile([P, 1024], F32, tag="mm", name="ps_qkt")
                for kp in range(KC // 2):
                    nc.tensor.matmul(
                        ps[:, 0:width],
                        w_sb[:, 2 * kp:2 * kp + 2, m * P:(m + 1) * P],
                        xt8_sb[:, nch, 2 * kp:2 * kp + 2, 0:width],
                        start=(kp == 0), stop=(kp == KC // 2 - 1),
                        perf_mode=DR)
                if which == 1 and m == 1:
                    nc.scalar.copy(dst[:, 0:width], ps[:, 0:width])
                else:
                    nc.scalar.copy(
                        dst[:, nch * 512:nch * 512 + width], ps[:, 0:width])

            def emit_v_near(mt):
                nlive = sum(1 for s in range(4) if mt < KBUD[s])
                psv = psA.tile([P, 1024], F32, tag="mm", name="ps_vn")
                for k in range(KC):
                    nc.tensor.matmul(
                        psv[:, 0:nlive * D],
                        xtf_sb[:, k, mt * P:(mt + 1) * P],
                        wv_sb[:, k, 0:nlive * D],
                        start=(k == 0), stop=(k == KC - 1))
                for s in range(nlive):
                    nc.scalar.mul(
                        vv16_sb[:, mt, s, 0:D], psv[:, s * D:(s + 1) * D],
                        vsc_sb[:, mt, s:s + 1])
                for s in range(min(nlive, 2)):
                    nc.vector.tensor_scalar_mul(
                        vv8_sb[:, mt, s, 0:D], psv[:, s * D:(s + 1) * D],
                        vsc_sb[:, mt, s:s + 1])

            def emit_v_far(mt):
                psv = psA.tile([P, 1024], F32, tag="mm", name="ps_vf")
                for k in range(KC):
                    nc.tensor.matmul(
                        psv[:, 0:D],
                        xt8_sb[:, mt // 4, k, (mt % 4) * P:(mt % 4 + 1) * P],
                        wv_sb[:, k, 0:D],
                        start=(k == 0), stop=(k == KC - 1))
                nc.scalar.mul(
                    vv8_sb[:, mt, 0, 0:D], psv[:, 0:D], vsc_sb[:, mt, 0:1])

            for nch in range(QCH):
                emit_qkt(0, 0, nch)
                emit_qkt(0, 1, nch)
                emit_qkt(1, 0, nch)
                if nch == 0:
                    emit_qkt(1, 1, 0, width=P)
            for mt in range(4):
                emit_v_near(mt)
            for mt in range(4, NT):
                emit_v_far(mt)

            # ---- phase 2: attention (qc-major so projection of finished
            # query chunks overlaps later chunks) with the classic pipeline:
            # S(i) matmuls, exp(i), PV(i-1); paired divides; proj interleave.
            strips = []
            for qc in range(QCH):
                for s in range(4):
                    kmax = min(KBUD[s], 4 * qc + 4)
                    for g in range((kmax + 1) // 2):
                        kts = [kt for kt in (2 * g, 2 * g + 1) if kt < kmax]
                        strips.append((s, qc, g, kts, kmax))

            opsums = {}
            pendings = []  # PV runs 2 strips behind its exp
            actions = []   # deferred (emit_at, fn) actions
            proj_done = [False] * NT

            def emit_pv(strip, pst):
                s, qc, g, kts, kmax = strip
                fp8 = (s <= 1 and qc >= 1)
                op = opsums[(s, qc)]
                if fp8:
                    nc.tensor.matmul(
                        op[:], vv8_sb[:, 2 * g:2 * g + 2, s, 0:65],
                        pst[:, 0:2, :],
                        start=(g == 0), stop=(2 * g + 2 >= kmax),
                        perf_mode=DR)
                else:
                    for d_, kt in enumerate(kts):
                        nc.tensor.matmul(
                            op[:], vv16_sb[:, kt, s, 0:65], pst[:, d_, :],
                            start=(kt == 0), stop=(kt == kmax - 1))

            def emit_divide(sa, qc):
                for s_ in (sa, sa + 1):
                    op = opsums.pop((s_, qc))
                    dh = rr.tile([1, 512], F16, tag="dh", name="dh")
                    nc.vector.tensor_copy(dh[:], op[64:65, :])
                    rb = psR.tile([D, 512], F32, tag="rb", name="rb")
                    nc.tensor.matmul(rb[:], ones_sb[:], dh[:],
                                     start=True, stop=True)
                    rbs = rbp.tile([D, 512], F32, tag="rbs", name="rbs")
                    nc.vector.reciprocal_approx_fast(rbs[:], rb[:])
                    base = (s_ % 2) * D
                    nc.vector.tensor_mul(
                        ot_sb[base:base + D, sa // 2, qc * 512:(qc + 1) * 512],
                        op[0:64, :], rbs[:])

            def emit_proj(mt):
                ps = psA.tile([P, 1024], F32, tag="mm", name="ps_proj")
                for nch2 in range(2):
                    for j in range(2):
                        nc.tensor.matmul(
                            ps[:, nch2 * 512:(nch2 + 1) * 512],
                            ot_sb[:, j, mt * P:(mt + 1) * P],
                            wp_sb[:, j, nch2 * 512:(nch2 + 1) * 512],
                            start=(j == 0), stop=(j == 1))
                yt = yp.tile([P, 1024], F16, tag="y", name="yt")
                if mt % 2 == 0:
                    nc.scalar.copy(yt[:], ps[:])
                else:
                    nc.vector.tensor_copy(yt[:], ps[:])
                nc.sync.dma_start(y_d[mt * P:(mt + 1) * P, :], yt[:])
                proj_done[mt] = True

            for i, strip in enumerate(strips):
                s, qc, g, kts, kmax = strip
                fp8 = (s <= 1 and qc >= 1)
                if (s, qc) not in opsums:
                    opsums[(s, qc)] = psO.tile([65, 512], F32, tag="o",
                                               name="opsum")
                if s < 2:
                    qt_t, kt_t = qt_sb[0], kt0_sb
                else:
                    qt_t, kt_t = qt_sb[1], kt1_sb
                base = (s % 2) * D
                w = len(kts)
                sps = psA.tile([P, 1024], F32, tag="mm", name="sps")
                for d_, kt in enumerate(kts):
                    nc.tensor.matmul(
                        sps[:, d_ * 512:(d_ + 1) * 512],
                        kt_t[base:base + D, kt * P:(kt + 1) * P],
                        qt_t[base:base + D, qc * 512:(qc + 1) * 512],
                        start=True, stop=True)
                pst = pp.tile([P, 2, 512], F8 if fp8 else F16,
                              tag="p8" if fp8 else "p16", name="pst")
                nc.scalar.activation(pst[:, 0:w, :], sps[:, 0:512 * w].rearrange(
                    "p (w n) -> p w n", n=512),
                    ExpF, bias=bias_sb[:, s:s + 1], scale=1.0 / 8192.0)
                if g == 2 * qc:
                    nc.vector.tensor_mul(
                        pst[:, 0:w, :],
                        pst[:, 0:w, :],
                        masks_sb[:, 0:512 * w].rearrange("p (w n) -> p w n", n=512))
                elif g == 2 * qc + 1:
                    nc.vector.tensor_mul(
                        pst[:, 0:w, :],
                        pst[:, 0:w, :],
                        masks_sb[:, 1024:1024 + 512 * w].rearrange(
                            "p (w n) -> p w n", n=512))
                while actions and actions[0][0] <= i:
                    actions.pop(0)[1]()
                if len(pendings) >= 2:
                    pstrip, ppst = pendings.pop(0)
                    emit_pv(pstrip, ppst)
                    ps_, qc_ = pstrip[0], pstrip[1]
                    nxt = pendings[0][0] if pendings else None
                    if (nxt is None or (nxt[0], nxt[1]) != (ps_, qc_)) \
                            and ps_ % 2 == 1:
                        actions.append((i + 1, (lambda a=ps_ - 1, b=qc_:
                                                emit_divide(a, b))))
                        if ps_ == 3:
                            mts = [4 * qc_, 4 * qc_ + 1, 4 * qc_ + 2, 4 * qc_ + 3]
                            for off, mt in enumerate(mts):
                                actions.append((i + 2 + 2 * off,
                                                (lambda m=mt: emit_proj(m))))
                pendings.append((strip, pst))
            for pstrip, ppst in pendings:
                emit_pv(pstrip, ppst)
            for _, fn in sorted(actions):  # flush (incl. divide(0,3))
                fn()
            emit_divide(2, 3)
            for mt in range(NT):
                if not proj_done[mt]:
                    emit_proj(mt)

    nc.compile()
    return nc


def _host_prep(x, w_qkv, w_proj):
    slopes = _slopes()
    in_maps = []
    xt_by_b = [np.ascontiguousarray(x[b].T) for b in range(B)]

    rr_ = np.arange(P)[:, None]
    cc = np.arange(512)[None, :]
    masks = np.concatenate(
        [(rr_ <= cc - P * d).astype(E4NP) for d in range(4)], axis=1)


    def chunk_major(a):
        # [C, N] -> [P, KC*N]: partition-major with contraction chunks inline
        n = a.shape[1]
        return np.ascontiguousarray(
            a.reshape(KC, P, n).transpose(1, 0, 2).reshape(P, KC * n))

    xt8_by_b, xtf_by_b = [], []
    for b in range(B):
        xt = xt_by_b[b]
        x8 = np.clip(xt, -240, 240).astype(E4NP)
        # [C, T] -> [P, QCH, KC, 512] (query-chunk major)
        xt8_by_b.append(np.ascontiguousarray(
            x8.reshape(KC, P, QCH, 512).transpose(1, 2, 0, 3).reshape(P, -1)))
        xtf_by_b.append(chunk_major(xt[:, 0:512].astype(np.float16)))

    group_data = []
    for g in range(4):
        H = [SLOT_HEADS[s][g] for s in range(4)]
        cols = np.concatenate([np.arange(h * D, (h + 1) * D) for h in H])
        wq8 = chunk_major((32.0 * w_qkv[:, cols]).astype(E4NP))
        wk8 = chunk_major((32.0 * w_qkv[:, C + cols]).astype(E4NP))
        wv = chunk_major(w_qkv[:, 2 * C + cols].astype(np.float16))
        wp = np.ascontiguousarray(
            w_proj[cols, :].astype(np.float16).reshape(2, P, C)
            .transpose(1, 0, 2).reshape(P, 2 * C))
        bias = np.stack(
            [-slopes[h] * np.arange(P, dtype=np.float64) for h in H],
            axis=1).astype(np.float32)
        vsc = np.broadcast_to(
            np.exp(-np.outer(128.0 * np.arange(NT),
                             np.array([slopes[h] for h in H]))
                   ).astype(np.float32).reshape(1, NT * 4),
            (P, NT * 4)).copy()
        group_data.append((wq8, wk8, wv, wp, bias, vsc))

    for c in range(N_CORES):
        b, g = divmod(c, 4)
        wq8, wk8, wv, wp, bias, vsc = group_data[g]
        in_maps.append({
            "xt8": xt8_by_b[b], "xtf": xtf_by_b[b],
            "wq8": wq8, "wk8": wk8, "wv": wv, "wp": wp,
            "masks": masks, "bias": bias, "vsc": vsc,
        })
    return in_maps


def kernel(x, w_qkv, w_proj):
    if "nc" not in _CACHE:
        _CACHE["nc"] = _build_program()
    nc = _CACHE["nc"]

    in_maps = _host_prep(np.asarray(x, np.float32), np.asarray(w_qkv, np.float32),
                         np.asarray(w_proj, np.float32))
    res = run_bass_kernel_spmd(nc, in_maps, list(range(N_CORES)), trace=TRACE)
    _CACHE["last_result"] = res

    y = np.zeros((B, T, C), dtype=np.float64)
    for c in range(N_CORES):
        b = c // 4
        y[b] += res.results[c]["y"].astype(np.float64)
    return y.astype(np.float32)
